# revision 1
# baseline (speedup 1.0000x reference)
"""Single-head attention (B=8, S=2048, DIN=DK=DV=1024) on 8 TRN2 NeuronCores.

Strategy: pure data-parallel — one batch element per core, identical SPMD
program, no collectives. All layout transposes are done host-side so the
device kernel is transpose-free.

Algebraic restructure (softmax is invariant to per-query constants):
    scores = (xq Wq^T + bq)(xk Wk^T + bk)^T / sqrt(dk)
           = xq M xk^T + e_s + (q-only terms that softmax cancels)
  with M = (Wq^T Wk) / sqrt(dk) folded on the host (weight-only precompute)
  and e_s[s] = xk[s] . (Wk^T bq) / sqrt(dk) computed on host per batch.
  This deletes the entire K projection from the device.

The scores matmul runs in fp8e4m3 with DoubleRow perf mode (2 contraction
rows per cycle): G is computed in bf16 (alpha=64 folded into M host-side so
the fp8 cast of G uses the normal range) and stored fp8; xk is cast fp8 on
the host. The exp activation applies scale=1/alpha to undo the fold.

The attention-value product is centered to absorb quantization noise:
    U = E @ V = (E-1) @ V + colsum(V),  r = sum_s E = 2048 + sum_s (E-1)
  colsum(V) = (sum_s xv) @ Wv^T is computed exactly on the host and enters
  the epilogue as a per-column constant; the (E-1) operand has ~3x smaller
  magnitude than E, shrinking both E- and V-side rounding noise.

  per core (feature-major layouts, features on partitions):
    GT[d,q]    = M^T.T @ xqT       (bf16, the only q-side projection) -> fp8
    V[s,v]     = xvT.T @ WvT       (bias bv folded into the epilogue)
    ST[s,q]    = xkT.T @ GT        (scores, fp8 DoubleRow, transposed layout)
    Em1[s,q]   = exp(ST/64 + e_s) - 1   (ACT exp then DVE subtract)
    U[q,v]     = Em1.T @ V         (centered unnormalized output)
    r[q]       = 2048 + Em1.T @ ones
    out[q,v]   = (U + cs) * (1/r) + bv  (two fused DVE passes per tile)

Matmuls run in bf16/fp8 (inputs pre-cast on host), fp32 accumulation in PSUM.
"""

import numpy as np
import ml_dtypes

import concourse.bass as bass
import concourse.tile as tile
from concourse import bacc, mybir
from concourse.bass_utils import run_bass_kernel_spmd

B, S, D = 8, 2048, 1024
N_CORES = 8
PB = 128           # partition block
NCH = 512          # matmul moving-dim / PSUM bank chunk
SB = S // PB       # 16 s-blocks
DB = D // PB       # 8 feature blocks
SCH = S // NCH     # 4 s-chunks
QCH = S // NCH     # 4 q-chunks
VCH = D // NCH     # 2 v-chunks
XSPLIT = ((0, 3), (3, 6), (6, 8))  # x-chunk DMA split across 3 engines
ALPHA = 64.0       # G scale folded into M host-side, undone in the exp
NF8 = 12           # s-blocks (of 16) whose U contribution runs fp8 DoubleRow

BF16 = mybir.dt.bfloat16
FP8 = mybir.dt.float8e4
F32 = mybir.dt.float32
DR = mybir.MatmulPerfMode.DoubleRow

_compiled = [None]


def _build():
    nc = bacc.Bacc("TRN2", target_bir_lowering=False, debug=False,
                   num_devices=N_CORES, num_swdge_queues=2,
                   enable_partition_id=False)

    # x*T chunks: [s_chunk][128 part(din)][din_blk][512 s]; per-partition row
    # of a chunk is contiguous for DMA efficiency.
    xqT = nc.dram_tensor("xqT", [SCH, PB, DB, NCH], BF16, kind="ExternalInput")
    xkT = nc.dram_tensor("xkT", [SCH, PB, DB, NCH], FP8, kind="ExternalInput")
    xvT = nc.dram_tensor("xvT", [SCH, PB, DB, NCH], BF16, kind="ExternalInput")
    # first VKB8 din-blocks of xv/Wv as fp8 (partial DoubleRow V projection)
    xvT8 = nc.dram_tensor("xvT8", [SCH, PB, 2, NCH], FP8, kind="ExternalInput")
    wv8T = nc.dram_tensor("wv8T", [PB, 2, D], FP8, kind="ExternalInput")
    # weight-like matrices, column-blocked: [out_blk][128 part(din)][din_blk][128 out]
    mT = nc.dram_tensor("mT", [DB, PB, DB, PB], BF16, kind="ExternalInput")
    wvT = nc.dram_tensor("wvT", [DB, PB, DB, PB], BF16, kind="ExternalInput")
    esT = nc.dram_tensor("esT", [PB, SB], F32, kind="ExternalInput")  # [p, s_blk]
    bv = nc.dram_tensor("bv", [1, D], F32, kind="ExternalInput")
    cs = nc.dram_tensor("cs", [1, D], F32, kind="ExternalInput")  # colsum(V)
    out = nc.dram_tensor("out", [S, D], F32, kind="ExternalOutput")

    with tile.TileContext(nc) as tc:
        with (
            tc.tile_pool(name="res", bufs=1) as res,      # phase-resident tensors
            tc.tile_pool(name="wpool", bufs=2) as wpool,  # streamed weights
            tc.tile_pool(name="xpool", bufs=3) as xpool,  # streamed x chunks
            tc.tile_pool(name="epool", bufs=2) as epool,  # exp-minus-1 tiles
            tc.tile_pool(name="epool8", bufs=2) as epool8,  # fp8 copy of Em1
            tc.tile_pool(name="etmp", bufs=3) as etmp,    # raw exp staging
            tc.tile_pool(name="opool", bufs=2) as opool,  # output staging
            tc.tile_pool(name="misc", bufs=1) as misc,
            tc.tile_pool(name="psA", bufs=2, space="PSUM") as psA,
            tc.tile_pool(name="psB", bufs=2, space="PSUM") as psB,
        ):
            dmae = [nc.sync, nc.gpsimd, nc.scalar]

            def load_x_chunk(xt_ap, xdram_sc, rot):
                for i, (a, b) in enumerate(XSPLIT):
                    dmae[(i + rot) % 3].dma_start(
                        out=xt_ap[:, a:b], in_=xdram_sc[:, a:b]
                    )

            # ---- constants ----
            bvB = misc.tile([PB, D], F32, tag="bvB")
            csB = misc.tile([PB, D], F32, tag="csB")
            ess = misc.tile([PB, SB], F32, tag="es")
            ones = misc.tile([PB, 1], BF16, tag="ones")
            nc.vector.memset(ones[:], 1.0)

            # ---- PE warmup: release the HAM clock throttle while the
            # startup DMAs are in flight (PE is otherwise idle ~7us) ----
            warm = misc.tile([PB, 256], BF16, tag="warm")
            nc.vector.memset(warm[:], 0.0)
            for i in range(24):
                pw = psB.tile([1, 256], F32, tag="pr")
                nc.tensor.matmul(out=pw[:], lhsT=ones[:], rhs=warm[:],
                                 start=True, stop=True)

            # ---- resident tensors ----
            GT = res.tile([PB, DB, S], FP8, tag="GT")    # [p(d), d_blk, q]
            XK = res.tile([PB, DB, S], FP8, tag="XK")    # [p(din), din_blk, s]
            V = res.tile([PB, SB, D], BF16, tag="V")     # [p(s), s_blk, v]
            V8 = res.tile([PB, NF8, D], FP8, tag="V8")   # fp8 copy, sb < NF8

            # ---- G projection (G = xq @ M, bf16), startup-critical ----
            # Cold-start choreography: the first PSUM group needs M.col0 +
            # all 8 kb-blocks of xq chunk 0 (~1.25MB); later groups need one
            # more 256KB M column each. Order the queues so data lands just
            # ahead of the PE.
            wt = wpool.tile([PB, DB, DB, PB], BF16, tag="w")
            nc.sync.dma_start(out=wt[:, 0], in_=mT[0])  # first column first
            for sc in range(SCH):
                xt = xpool.tile([PB, DB, NCH], BF16, tag="x")
                if sc == 0:
                    nc.scalar.dma_start(out=xt[:, 0:3], in_=xqT[0][:, 0:3])
                    nc.sync.dma_start(out=xt[:, 3:6], in_=xqT[0][:, 3:6])
                    nc.gpsimd.dma_start(out=xt[:, 6:8], in_=xqT[0][:, 6:8])
                    for db in range(1, DB):
                        dmae[(db % 2) * 2].dma_start(out=wt[:, db], in_=mT[db])
                    # startup-noncritical loads, behind the first blocks
                    nc.gpsimd.dma_start(out=ess[:], in_=esT[:])
                    nc.gpsimd.dma_start(out=bvB[:], in_=bv.ap().to_broadcast((PB, D)))
                    nc.gpsimd.dma_start(out=csB[:], in_=cs.ap().to_broadcast((PB, D)))
                else:
                    load_x_chunk(xt, xqT[sc], rot=sc)
                for db in range(DB):
                    pt = psA.tile([PB, NCH], F32, tag="pp")
                    for kb in range(DB):
                        nc.tensor.matmul(
                            out=pt[:],
                            lhsT=wt[:, db, kb, :],
                            rhs=xt[:, kb, :],
                            start=(kb == 0),
                            stop=(kb == DB - 1),
                        )
                    nc.vector.tensor_copy(
                        out=GT[:, db, sc * NCH:(sc + 1) * NCH], in_=pt[:]
                    )

            # ---- load xk directly (no K projection), fp8 ----
            for sc in range(SCH):
                load_x_chunk(XK[:, :, sc * NCH:(sc + 1) * NCH], xkT[sc], rot=sc)

            # ---- projection V (input is stationary, weight is moving) ----
            # din blocks 0-1 run fp8 DoubleRow; the bf16 weights carry a
            # gamma=32 scale (so Wv lands in fp8 normal range) undone in the
            # PSUM->SBUF copy.
            wt = wpool.tile([PB, DB, DB, PB], BF16, tag="w")
            for db in range(DB):
                dmae[db % 2].dma_start(out=wt[:, db], in_=wvT[db])
            wv8 = misc.tile([PB, 2, D], FP8, tag="wv8")
            nc.gpsimd.dma_start(out=wv8[:], in_=wv8T.ap())
            for sc in range(SCH):
                xt = xpool.tile([PB, DB, NCH], BF16, tag="x")
                load_x_chunk(xt, xvT[sc], rot=sc)
                xt8 = xpool.tile([PB, 2, NCH], FP8, tag="x8")
                nc.gpsimd.dma_start(out=xt8[:], in_=xvT8[sc])
                for sbl in range(NCH // PB):  # s-blocks within this chunk
                    sb = sc * (NCH // PB) + sbl
                    for vc in range(VCH):
                        pt = psA.tile([PB, NCH], F32, tag="pp")
                        nc.tensor.matmul(
                            out=pt[:],
                            lhsT=xt8[:, 0:2, sbl * PB:(sbl + 1) * PB],
                            rhs=wv8[:, 0:2, vc * NCH:(vc + 1) * NCH],
                            start=True,
                            stop=False,
                            perf_mode=DR,
                        )
                        for kb in range(2, DB):
                            nc.tensor.matmul(
                                out=pt[:],
                                lhsT=xt[:, kb, sbl * PB:(sbl + 1) * PB],
                                rhs=wt[:, 4 * vc:4 * (vc + 1), kb, :],
                                start=False,
                                stop=(kb == DB - 1),
                            )
                        nc.vector.tensor_scalar_mul(
                            V[:, sb, vc * NCH:(vc + 1) * NCH], pt[:],
                            1.0 / 32.0,
                        )
                        if sb < NF8:
                            nc.vector.tensor_copy(
                                out=V8[:, sb, vc * NCH:(vc + 1) * NCH],
                                in_=V[:, sb, vc * NCH:(vc + 1) * NCH],
                            )

            # ---- attention, per q-chunk of 512 ----
            for qc in range(QCH):
                q0 = qc * NCH
                # scores^T [s, q-chunk] (fp8 DoubleRow), exp, minus 1 -> Em1
                Em1 = epool.tile([PB, SB, NCH], BF16, tag="E")
                Em18 = epool8.tile([PB, NF8, NCH], FP8, tag="E8")
                for sb in range(SB):
                    pt = psA.tile([PB, NCH], F32, tag="ps")
                    for kb in range(DB // 2):
                        nc.tensor.matmul(
                            out=pt[:],
                            lhsT=XK[:, 2 * kb:2 * kb + 2, sb * PB:(sb + 1) * PB],
                            rhs=GT[:, 2 * kb:2 * kb + 2, q0:q0 + NCH],
                            start=(kb == 0),
                            stop=(kb == DB // 2 - 1),
                            perf_mode=DR,
                        )
                    et = etmp.tile([PB, NCH], BF16, tag="et")
                    nc.scalar.activation(
                        out=et[:], in_=pt[:],
                        func=mybir.ActivationFunctionType.Exp,
                        bias=ess[:, sb:sb + 1],
                        scale=1.0 / ALPHA,
                    )
                    nc.vector.tensor_scalar_sub(Em1[:, sb, :], et[:], 1.0)
                    if sb < NF8:
                        nc.vector.tensor_copy(
                            out=Em18[:, sb, :], in_=Em1[:, sb, :]
                        )
                # per q-block of 128: denominators r, then U, then epilogue
                for qb in range(NCH // PB):
                    eq = slice(qb * PB, (qb + 1) * PB)
                    pr = psB.tile([PB, 1], F32, tag="pr")
                    for sb in range(SB):
                        nc.tensor.matmul(
                            out=pr[:], lhsT=Em1[:, sb, eq], rhs=ones[:],
                            start=(sb == 0), stop=(sb == SB - 1),
                        )
                    rden = misc.tile([PB, 1], F32, tag="rden")
                    nc.vector.tensor_scalar_add(rden[:], pr[:], float(S))
                    recip = misc.tile([PB, 1], F32, tag="recip")
                    nc.vector.reciprocal(out=recip[:], in_=rden[:])
                    qrow = q0 + qb * PB
                    last = (qc == QCH - 1) and (qb == NCH // PB - 1)
                    if last:
                        # precompute cs*(1/r)+bv while the U matmuls run, so
                        # each post-matmul tail piece needs a single stt
                        crb = opool.tile([PB, D], F32, tag="crb")
                        for vc in range(VCH):
                            nc.vector.scalar_tensor_tensor(
                                out=crb[:, vc * NCH:(vc + 1) * NCH],
                                in0=csB[:, vc * NCH:(vc + 1) * NCH],
                                scalar=recip[:],
                                in1=bvB[:, vc * NCH:(vc + 1) * NCH],
                                op0=mybir.AluOpType.mult,
                                op1=mybir.AluOpType.add,
                            )
                    for vc in range(VCH):
                        pu = psB.tile([PB, NCH], F32, tag="pu")
                        for kb in range(NF8 // 2):  # fp8 DoubleRow part
                            nc.tensor.matmul(
                                out=pu[:],
                                lhsT=Em18[:, 2 * kb:2 * kb + 2, eq],
                                rhs=V8[:, 2 * kb:2 * kb + 2,
                                       vc * NCH:(vc + 1) * NCH],
                                start=(kb == 0),
                                stop=False,
                                perf_mode=DR,
                            )
                        for sb in range(NF8, SB):   # bf16 tail
                            nc.tensor.matmul(
                                out=pu[:],
                                lhsT=Em1[:, sb, eq],
                                rhs=V[:, sb, vc * NCH:(vc + 1) * NCH],
                                start=False,
                                stop=(sb == SB - 1),
                            )
                        ot = opool.tile([PB, NCH], F32, tag="ot")
                        # the very last block pipelines finer stt->DMA pieces
                        # so the kernel tail shortens
                        npc = 4 if last else 2
                        h = NCH // npc
                        c0 = vc * NCH
                        for p in range(npc):
                            a = p * h
                            if last:  # single-stt tail: U*(1/r) + crb
                                nc.vector.scalar_tensor_tensor(
                                    out=ot[:, a:a + h],
                                    in0=pu[:, a:a + h],
                                    scalar=recip[:],
                                    in1=crb[:, c0 + a:c0 + a + h],
                                    op0=mybir.AluOpType.mult,
                                    op1=mybir.AluOpType.add,
                                )
                                eng = dmae[(vc * npc + p) % 3]
                            else:
                                # (U + cs) * (1/r) + bv, two fused DVE passes
                                nc.vector.tensor_tensor(
                                    out=ot[:, a:a + h],
                                    in0=pu[:, a:a + h],
                                    in1=csB[:, c0 + a:c0 + a + h],
                                    op=mybir.AluOpType.add,
                                )
                                nc.vector.scalar_tensor_tensor(
                                    out=ot[:, a:a + h],
                                    in0=ot[:, a:a + h],
                                    scalar=recip[:],
                                    in1=bvB[:, c0 + a:c0 + a + h],
                                    op0=mybir.AluOpType.mult,
                                    op1=mybir.AluOpType.add,
                                )
                                eng = nc.sync if p % 2 == 0 else nc.scalar
                            eng.dma_start(
                                out=out[qrow:qrow + PB, c0 + a:c0 + a + h],
                                in_=ot[:, a:a + h],
                            )

    nc.compile()
    return nc


def _prep_host(query, key_, value, Wq_w, Wq_b, Wk_w, Wk_b, Wv_w, Wv_b):
    """Host-side sharding, layout marshalling, and weight-only algebra."""
    bf16 = ml_dtypes.bfloat16
    fp8 = ml_dtypes.float8_e4m3
    scale = np.float32(1.0 / np.sqrt(D))

    def prep_x(x, dt):  # [S, D] fp32 -> [SCH, PB, DB, NCH] (x.T, blocked)
        # xc[sc, p, kb, n] = x[sc*NCH+n, kb*PB+p]
        return np.ascontiguousarray(
            x.reshape(SCH, NCH, DB, PB).transpose(0, 3, 2, 1)
        ).astype(dt)

    def prep_w(wT):  # [Din, Dout] fp32 -> [DB, PB, DB, PB] bf16 (blocked)
        # wc[db, p, kb, j] = wT[kb*PB+p, db*PB+j]
        return np.ascontiguousarray(
            wT.reshape(DB, PB, DB, PB).transpose(2, 1, 0, 3)
        ).astype(bf16)

    # weight-only precompute: M = (Wq^T Wk) * scale * ALPHA  [din, din]
    M = (Wq_w.T @ Wk_w) * (scale * np.float32(ALPHA))
    mc = prep_w(M)
    wv = prep_w(np.ascontiguousarray(Wv_w.T) * np.float32(32.0))
    wv8 = np.ascontiguousarray(
        (Wv_w.T[:256] * np.float32(32.0)).reshape(2, PB, D).transpose(1, 0, 2)
    ).astype(fp8)
    c = (Wk_w.T @ Wq_b) * scale  # [din]; e_s = xk @ c
    bvr = np.ascontiguousarray(Wv_b.reshape(1, D)).astype(np.float32)

    in_maps = []
    for b in range(B):
        e_s = (key_[b] @ c).astype(np.float32)  # [S]
        # colsum(V) = (sum_s xv) @ Wv^T, exact in float64
        csv = (value[b].astype(np.float64).sum(axis=0)
               @ Wv_w.T.astype(np.float64)).astype(np.float32)
        in_maps.append({
            "xqT": prep_x(query[b], bf16),
            "xkT": prep_x(key_[b], fp8),
            "xvT": prep_x(value[b], bf16),
            "xvT8": np.ascontiguousarray(
                value[b].reshape(SCH, NCH, DB, PB)[:, :, 0:2, :]
                .transpose(0, 3, 2, 1)
            ).astype(fp8),
            "mT": mc, "wvT": wv, "wv8T": wv8,
            "esT": np.ascontiguousarray(e_s.reshape(SB, PB).T),
            "bv": bvr,
            "cs": np.ascontiguousarray(csv.reshape(1, D)),
        })
    return in_maps


def kernel(query, key_, value, Wq_w, Wq_b, Wk_w, Wk_b, Wv_w, Wv_b):
    query = np.asarray(query, np.float32)
    key_ = np.asarray(key_, np.float32)
    value = np.asarray(value, np.float32)
    Wq_w = np.asarray(Wq_w, np.float32)
    Wq_b = np.asarray(Wq_b, np.float32)
    Wk_w = np.asarray(Wk_w, np.float32)
    Wk_b = np.asarray(Wk_b, np.float32)
    Wv_w = np.asarray(Wv_w, np.float32)
    Wv_b = np.asarray(Wv_b, np.float32)

    if _compiled[0] is None:
        _compiled[0] = _build()
    nc = _compiled[0]

    in_maps = _prep_host(query, key_, value, Wq_w, Wq_b, Wk_w, Wk_b, Wv_w, Wv_b)
    last_err = None
    for attempt in range(3):
        try:
            res = run_bass_kernel_spmd(nc, in_maps, list(range(N_CORES)))
            out = np.stack([res.results[i]["out"] for i in range(N_CORES)], axis=0)
            if np.isfinite(out).all():
                return out
            last_err = RuntimeError("non-finite values in device output")
        except Exception as e:  # transient device errors (e.g. NRT exec unit)
            last_err = e
    raise last_err

